# revision 21
# baseline (speedup 1.0000x reference)
"""Trainium2 Bass kernel: batched single-channel 3x3 valid conv, 16 output channels.

reference: x [32, 512, 512] f32, kernels [16, 3, 3] f32
           -> out [32, 16, 510, 510] f32  (cross-correlation, VALID, stride 1)

Strategy (memory-regime problem: output is 532 MB, input 33 MB):
  - Data-parallel: 4 images per core across 8 cores; kernels replicated.
  - 30-row output blocks (510 = 17 x 30): per block one PE matmul per
    channel-group with contraction K = 3 column-shifts x 32 input rows = 96
    against a host-precomputed banded lhsT [96, 120] (M = 4 channels x 30
    rows). Inputs/weights in bf16 (1 PE cycle/row vs 4 for fp32; harness
    rel-err gate is 2e-2, bf16 conv lands ~2e-3) accumulated in f32 PSUM.
  - The output DRAM tensor uses a device-friendly layout
    [b, group, 120, 17, 510] so each (image, group) flush is ONE
    120-partition 4.2 MB dma_start (16 output DMAs per core total; the
    ~670us baseline used 128 calls x 551 KB on 30 partitions, which idles
    half the SDMA engines and pays ~1.5 us per-call overhead 8x more
    often). Host reassembles with a cheap transpose.
  - Input loads: 3 dma_starts per image (one per column-shift dx, each
    filling 32 partitions of the [96, 17*510] rhs tile) on the gpsimd
    (SWDGE) ring, keeping both HWDGE rings and the ACT sequencer free --
    issuing a dma_start costs ~1.2 us of sequencer time, so the old
    9-loads-per-half on nc.scalar serialized ~85 us of issue work with
    the ACT engine's PSUM->SBUF copies.
  - PSUM->SBUF staging copies alternate DVE/ACT; output flushes on the SP
    (nc.sync) HWDGE ring.
"""

import numpy as np
import ml_dtypes

import concourse.bass as bass
import concourse.mybir as mybir
import concourse.tile as tile
from concourse import bacc
from concourse.bass_utils import run_bass_kernel_spmd

N_CORES = 8
B, H, W = 32, 512, 512
KN, KS = 16, 3
OH, OW = H - KS + 1, W - KS + 1  # 510, 510
B_LOC = B // N_CORES  # 4

ROWS = 30                # output rows per block (510 = 17 * 30)
IN_ROWS = ROWS + KS - 1  # 32 input rows per block
KDIM = KS * IN_ROWS      # 96 contraction
NBLK = OH // ROWS        # 17
KG = 4                   # channels per matmul group
N_GROUPS = KN // KG      # 4
M = KG * ROWS            # 120 psum partitions

F32 = mybir.dt.float32
DT_MAP = {
    "bf16": (mybir.dt.bfloat16, ml_dtypes.bfloat16),
    "f32r": (mybir.dt.float32r, np.float32),
    "f32": (mybir.dt.float32, np.float32),
}
DTYPE = "bf16"
# device-side output storage: bf16 halves the dominant HBM stream (the
# 532 MB output); PSUM accumulation stays f32 and the host upconverts.
# Total rel err ~5e-3 vs the 2e-2 harness gate.
OUT_DTYPE = "bf16"
TWO_RINGS = False  # flush even groups on SP ring, odd groups on ACT ring
COPY_ENGINES = 2   # 2 = DVE/ACT alternate; 3 = DVE/ACT/Pool
PAIR_COPIES = False  # one PSUM->SBUF copy per 2 blocks (2 PSUM banks)
BS = 512           # block slot stride in PSUM/stage when PAIR_COPIES
GROUPS_PER_FLUSH = 2  # 1, 2, or 4 groups share one stage tile / flush DMA
IN_BUFS = 2
STAGE_BUFS = 3


def _build_nc(
    dtype=None,
    reps=1,
    out_dtype=None,
    two_rings=None,
    copy_engines=None,
    pair_copies=None,
    gpf=None,
):
    in_dt = DT_MAP[dtype or DTYPE][0]
    out_dt = DT_MAP[out_dtype or OUT_DTYPE][0]
    two_rings = TWO_RINGS if two_rings is None else two_rings
    copy_engines = COPY_ENGINES if copy_engines is None else copy_engines
    pair_copies = PAIR_COPIES if pair_copies is None else pair_copies
    gpf = GROUPS_PER_FLUSH if gpf is None else gpf
    nc = bacc.Bacc("TRN2", target_bir_lowering=False, debug=False)
    x_t = nc.dram_tensor("x", [B_LOC, H, W], in_dt, kind="ExternalInput")
    w_t = nc.dram_tensor("w", [KDIM, N_GROUPS * M], in_dt, kind="ExternalInput")
    # device layout: [b, g, p=(k,y), blk, x]; host transposes to [b,ch,510,510]
    out_t = nc.dram_tensor(
        "out", [B_LOC, N_GROUPS, M, NBLK, OW], out_dt, kind="ExternalOutput"
    )

    with tile.TileContext(nc) as tc:
        with (
            tc.tile_pool(name="wpool", bufs=1) as wpool,
            tc.tile_pool(name="inpool", bufs=IN_BUFS) as inpool,
            tc.tile_pool(name="psum", bufs=8, space="PSUM") as psum_pool,
            tc.tile_pool(name="stage", bufs=STAGE_BUFS) as stage_pool,
        ):
            wt = wpool.tile([KDIM, N_GROUPS * M], in_dt)
            nc.sync.dma_start(out=wt[:, :], in_=w_t[:, :])
            cp = 0
            for b in [b for _ in range(reps) for b in range(B_LOC)]:
                # whole-image rhs tile; partition p = (dx, y'), free = (blk, x):
                # base[dx*32 + y', blk*510 + x] = x[b, blk*30 + y', x + dx]
                base = inpool.tile(
                    [KDIM, NBLK * OW], in_dt, name="base", tag="base"
                )
                src = x_t.ap()[b]  # [H, W]
                for dx in range(KS):
                    nc.gpsimd.dma_start(
                        out=base[dx * IN_ROWS : (dx + 1) * IN_ROWS, :],
                        in_=bass.AP(
                            src.tensor,
                            src.offset + dx,
                            [[W, IN_ROWS], [ROWS * W, NBLK], [1, OW]],
                        ),
                    )
                for g in range(N_GROUPS):
                    def do_copy(dst, src_ap, k):
                        w_cp = k % copy_engines
                        if w_cp == 0:
                            nc.vector.tensor_copy(out=dst, in_=src_ap)
                        elif w_cp == 1:
                            nc.scalar.copy(out=dst, in_=src_ap)
                        else:
                            nc.gpsimd.tensor_copy(out=dst, in_=src_ap)

                    lhsT = wt[:, g * M : (g + 1) * M]
                    if pair_copies:
                        # blocks live at 512-elem slots (PSUM-bank aligned,
                        # 2 pad cols); one copy moves 2 blocks' banks
                        big = stage_pool.tile(
                            [M, NBLK * BS], out_dt, name="big", tag="big"
                        )
                        for jp in range(0, NBLK, 2):
                            npair = min(2, NBLK - jp)
                            ps = psum_pool.tile(
                                [M, npair * BS],
                                F32,
                                name="ps",
                                tag=f"ps{npair}",
                                bufs=3 if npair == 2 else 2,
                            )
                            for q in range(npair):
                                nc.tensor.matmul(
                                    ps[:, q * BS : q * BS + OW],
                                    lhsT=lhsT,
                                    rhs=base[:, (jp + q) * OW : (jp + q + 1) * OW],
                                    start=True,
                                    stop=True,
                                )
                            do_copy(
                                big[:, jp * BS : (jp + npair) * BS], ps[:, :], cp
                            )
                            cp += 1
                        src = big[:, :].rearrange("p (blk x) -> p blk x", x=BS)[
                            :, :, 0:OW
                        ]
                    else:
                        # gpf groups share one stage tile; one flush DMA per
                        # gpf groups: DRAM AP [p:120, g:gpf, (blk x):8670]
                        # (3 dims; partition dim between two free dims is
                        # fine -- the AP only needs matching iteration order)
                        if g % gpf == 0:
                            big = stage_pool.tile(
                                [M, gpf * NBLK * OW], out_dt, name="big", tag="big"
                            )
                        off = (g % gpf) * NBLK * OW
                        for j in range(NBLK):
                            ps = psum_pool.tile([M, OW], F32)
                            nc.tensor.matmul(
                                ps[:, :],
                                lhsT=lhsT,
                                rhs=base[:, j * OW : (j + 1) * OW],
                                start=True,
                                stop=True,
                            )
                            do_copy(
                                big[:, off + j * OW : off + (j + 1) * OW],
                                ps[:, :],
                                cp,
                            )
                            cp += 1
                        if g % gpf == gpf - 1:
                            if gpf == 1:
                                view = out_t[b, g, :, :, :]
                            else:
                                view = out_t[
                                    b, g - gpf + 1 : g + 1, :, :, :
                                ].rearrange("g p blk x -> p g (blk x)")
                            ring = (
                                nc.scalar
                                if (two_rings and (g // gpf) % 2)
                                else nc.sync
                            )
                            ring.dma_start(out=view, in_=big[:, :])
    nc.finalize()
    return nc


def _pack_weights(kernels: np.ndarray, dtype=None) -> np.ndarray:
    """lhsT pack: w[dx*IN_ROWS + y + dy, g*M + k*ROWS + y] = kernels[g*KG+k, dy, dx].

    psum[k*ROWS + y, n] = sum_{dx, y'} lhsT[dx*IN_ROWS + y', k*ROWS + y]
                                       * x[r + y', n + dx]
                        = sum_{dy, dx} kernels[g*KG+k, dy, dx] * x[r + y + dy, n + dx]
    """
    w = np.zeros((KDIM, N_GROUPS * M), np.float32)
    y = np.arange(ROWS)
    for g in range(N_GROUPS):
        for dx in range(KS):
            for k in range(KG):
                for dy in range(KS):
                    w[dx * IN_ROWS + y + dy, g * M + k * ROWS + y] = kernels[
                        g * KG + k, dy, dx
                    ]
    return w.astype(DT_MAP[dtype or DTYPE][1])


def _prep_in_maps(x, kernels, dtype=None):
    np_dt = DT_MAP[dtype or DTYPE][1]
    x = np.ascontiguousarray(np.asarray(x, dtype=np.float32)).astype(np_dt)
    wp = _pack_weights(np.asarray(kernels, dtype=np.float32), dtype)
    return [
        {"x": x[c * B_LOC : (c + 1) * B_LOC], "w": wp} for c in range(N_CORES)
    ]


def _assemble(cores_out):
    # [cores*B_LOC, g, (k,y), blk, x] -> [B, (g,k), (blk,y), x], f32
    arr = np.concatenate(cores_out, axis=0)
    arr = arr.reshape(B, N_GROUPS, KG, ROWS, NBLK, OW)
    arr = arr.transpose(0, 1, 2, 4, 3, 5).astype(np.float32)
    return np.ascontiguousarray(arr).reshape(B, KN, OH, OW)


def run(x, kernels, trace=False, dtype=None, **spmd_kwargs):
    assert np.asarray(x).shape == (B, H, W)
    assert np.asarray(kernels).shape == (KN, KS, KS)
    nc = _build_nc(dtype)
    in_maps = _prep_in_maps(x, kernels, dtype)
    res = run_bass_kernel_spmd(
        nc, in_maps, core_ids=list(range(N_CORES)), trace=trace, **spmd_kwargs
    )
    out = _assemble([res.results[c]["out"] for c in range(N_CORES)])
    return out, res


def kernel(x, kernels):
    out, _ = run(x, kernels, trace=False)
    return out
